# revision 1
# baseline (speedup 1.0000x reference)
"""DiT block with block-diffusion sparse attention on 8 Trainium2 NeuronCores.

Strategy:
  - adaLN modulation: replicated on every core (tiny matmul).
  - LN1 + modulate: replicated (token-major, bn_stats), h cast to bf16,
    bounced through DRAM and DMA-transposed to feature-major h^T.
  - QKV + RoPE + attention: head-parallel, 2 heads per core. Scores are
    computed transposed (S^T[k, q]) so the softmax denominator falls out of
    the AV matmul via a ones-column appended to V; only the non-masked
    block-sparse k-ranges are computed.
  - AllToAll converts head-sharded attention output to token-sharded
    (256 tokens per core).
  - attn_out projection, LN2, and the 4x MLP run token-sharded with full
    (bf16) weights; no further collectives.
All matmuls run in bf16 with fp32 accumulation; norms/softmax stats in fp32.
"""

import os
import numpy as np
import ml_dtypes

import concourse.bass as bass
import concourse.tile as tile
from concourse import bacc, mybir
from concourse.bass_utils import run_bass_kernel_spmd
from concourse.masks import make_identity

bf16 = ml_dtypes.bfloat16
FP = mybir.dt.float32
BF = mybir.dt.bfloat16
AF = mybir.ActivationFunctionType
ALU = mybir.AluOpType

NCORES = 8
S, N, D, H, HD, BS, COND = 2048, 1024, 1024, 16, 64, 16, 128
TOK = S // NCORES  # 256 tokens per core after A2A


def _attn_schedule():
    """Per q-chunk list of (ktile, col0, col1, mask) in S^T orientation."""
    sched = []
    for c in range(4):
        items = []
        if c < 2:  # noisy q chunk
            for j in range(4 * c + 4):  # clean k tiles, bq > bk
                js = j - 4 * c
                if js < 0:
                    items.append((8 + j, 0, 512, None))
                else:
                    items.append((8 + j, 128 * js, 512, "strict"))
            for s in range(4):  # own-block diagonal (noisy k)
                items.append((4 * c + s, 128 * s, 128 * s + 128, "diag"))
        else:  # clean q chunk, bq >= bk
            cq = c - 2
            for j in range(4 * cq + 4):
                js = j - 4 * cq
                if js < 0:
                    items.append((8 + j, 0, 512, None))
                else:
                    items.append((8 + j, 128 * js, 512, "incl"))
        assert items[0][1] == 0 and items[0][2] == 512
        sched.append(items)
    return sched


MASK_OFF = {"diag": 0, "strict": 128, "incl": 256}


def build_program(single=False):
    """single=True builds a 1-device variant (A2A replaced by a local DMA
    copy) for TimelineSim cost-model analysis."""
    nc = bacc.Bacc(
        "TRN2", target_bir_lowering=False, debug=False,
        enable_asserts=False, num_devices=1 if single else NCORES,
    )

    def din(name, shape, dt=FP):
        return nc.dram_tensor(name, shape, dt, kind="ExternalInput").ap()

    x_d = din("x", [S, D], BF)
    xs_d = din("xslice", [TOK, D], BF)
    c_d = din("cvec", [COND, 1])
    wqkv_d = din("wqkvT", [3, 128, 8, 128], BF)       # (s, p, k, c)
    wao_d = din("waoT", [2, 128, 4, 8, 128], BF)      # (g, p, mi, k, c)
    w1_d = din("w1T", [8, 128, 4, 8, 128], BF)        # (g, p, mi, k, c)
    w2_d = din("w2T", [8, 128, 32, 128], BF)          # (m, p, k, c)
    adaw_d = din("adawT", [COND, 6 * D], BF)
    adab_sh_d = din("adab_sh", [1, D])
    adab_sc_d = din("adab_sc", [1, D])
    n1w_d = din("n1w", [1, D])
    smallc_d = din("smallc", [128, 80])  # n2w 0:8 | b1 8:40 | b2 40:48 | adab_scat 48:80
    trig_d = din("trig", [128, 2 * N], BF)            # cos | sin(signed)
    mask01_d = din("mask01", [128, 384], BF)          # diag | strict | incl
    out_d = nc.dram_tensor("out", [TOK, D], FP, kind="ExternalOutput").ap()

    sched = _attn_schedule()

    with tile.TileContext(nc) as tc:
        with tc.tile_pool(name="const", bufs=1) as const, \
             tc.tile_pool(name="dram", bufs=1, space="DRAM") as dram, \
             tc.tile_pool(name="rows", bufs=1) as rows, \
             tc.tile_pool(name="qkvr", bufs=1) as qkvr, \
             tc.tile_pool(name="vaugp", bufs=1) as vaugp, \
             tc.tile_pool(name="x2p", bufs=1) as x2p, \
             tc.tile_pool(name="gp", bufs=1) as gp, \
             tc.tile_pool(name="tps", bufs=2, space="PSUM") as tps:

            # ---------------- constants / small inputs ----------------
            trig_sb = const.tile([128, 2 * N], BF)
            nc.sync.dma_start(out=trig_sb, in_=trig_d)
            cos_sb = trig_sb[:, 0:N]
            sin_sb = trig_sb[:, N:2 * N]
            mask_sb = const.tile([128, 384], BF)
            nc.sync.dma_start(out=mask_sb, in_=mask01_d)
            smallc = const.tile([128, 80], FP)
            nc.sync.dma_start(out=smallc, in_=smallc_d)
            n2w_sb = smallc[:, 0:8]
            b1_sb = smallc[:, 8:40]
            b2_sb = smallc[:, 40:48]
            adab_scat = smallc[:, 48:80]
            adab_sh_sb = const.tile([1, D], FP)
            nc.sync.dma_start(out=adab_sh_sb, in_=adab_sh_d)
            adab_sc_sb = const.tile([1, D], FP)
            nc.sync.dma_start(out=adab_sc_sb, in_=adab_sc_d)
            n1w_sb = const.tile([1, D], FP)
            nc.sync.dma_start(out=n1w_sb, in_=n1w_d)
            cf_sb = const.tile([COND, 1], FP)
            nc.sync.dma_start(out=cf_sb, in_=c_d)
            c_sb = const.tile([COND, 1], BF)
            nc.vector.tensor_copy(out=c_sb, in_=cf_sb)
            ones_sb = const.tile([128, 1], BF)
            nc.vector.memset(ones_sb, 1.0)
            eps128 = const.tile([128, 1], FP)
            nc.vector.memset(eps128, 1e-5)
            eps1 = const.tile([1, 1], FP)
            nc.vector.memset(eps1, 1e-5)
            ident_f = const.tile([128, 128], FP)
            make_identity(nc, ident_f)
            ident_b = const.tile([128, 128], BF)
            nc.vector.tensor_copy(out=ident_b, in_=ident_f)

            # ---------------- phase 0: adaLN modulation ----------------
            mods_dr = dram.tile([12, 512], FP)
            with tc.tile_pool(name="adaw", bufs=1) as adawp, \
                 tc.tile_pool(name="mrow", bufs=3) as mrowp, \
                 tc.tile_pool(name="modsps", bufs=2, space="PSUM") as modsps:
                adaw_sb = adawp.tile([COND, 6 * D], BF)
                nc.scalar.dma_start(out=adaw_sb, in_=adaw_d)
                for t in range(12):
                    ps = modsps.tile([1, 512], FP, tag="modps")
                    nc.tensor.matmul(ps, c_sb, adaw_sb[:, 512 * t:512 * t + 512],
                                     start=True, stop=True)
                    mrow = mrowp.tile([1, 512], FP, tag="mrow")
                    nc.vector.tensor_copy(out=mrow, in_=ps)
                    nc.sync.dma_start(out=mods_dr[t], in_=mrow)

            mods_flat = mods_dr.rearrange("a b -> (a b)")
            # scatter [1024] vectors to [128, 8] via PE transpose of [8, 128]
            scat = []
            with tc.tile_pool(name="scw", bufs=2) as scw:
                for i, nm in enumerate(("gmsa", "shmlp", "scmlp", "gmlp")):
                    off = (2 + i) * D
                    w8 = scw.tile([8, 128], FP, tag="scw")
                    nc.sync.dma_start(
                        out=w8,
                        in_=mods_flat[off:off + D].rearrange("(i p) -> i p", i=8))
                    ps = tps.tile([128, 8], FP, tag="tp")
                    nc.tensor.transpose(ps, w8, ident_f[0:8, 0:8])
                    t = rows.tile([128, 8], FP, name=f"scat_{nm}")
                    nc.vector.tensor_add(t, ps, adab_scat[:, 8 * i:8 * i + 8])
                    scat.append(t)
            scat_gmsa, scat_shmlp, scat_scmlp, scat_gmlp = scat

            # LN1 gamma/beta: folded into the qkv weights (h only feeds qkv):
            # qkv = hhat @ (gamma*W)^T + beta@W^T. Build gamma/beta as
            # [128, 8] per-partition scalars via the DRAM+PE-transpose route.
            rows2_dr = dram.tile([2, D], FP)
            with tc.tile_pool(name="rowtmp", bufs=1) as rowtmp:
                sh_row = rowtmp.tile([1, D], FP)
                nc.sync.dma_start(out=sh_row, in_=mods_flat[0:D][None, :])
                nc.vector.tensor_add(sh_row, sh_row, adab_sh_sb)
                sc_row = rowtmp.tile([1, D], FP)
                nc.sync.dma_start(out=sc_row, in_=mods_flat[D:2 * D][None, :])
                nc.vector.tensor_add(sc_row, sc_row, adab_sc_sb)
                g1row = rowtmp.tile([1, D], FP)
                nc.vector.tensor_scalar_add(g1row, sc_row, 1.0)
                nc.vector.tensor_mul(g1row, g1row, n1w_sb)
                nc.sync.dma_start(out=rows2_dr[0], in_=g1row)
                nc.sync.dma_start(out=rows2_dr[1], in_=sh_row)
            with tc.tile_pool(name="scw2", bufs=2) as scw2:
                ln1scat = []
                for i in range(2):
                    w8 = scw2.tile([8, 128], FP, tag="scw2")
                    nc.sync.dma_start(
                        out=w8, in_=rows2_dr[i].rearrange("(i p) -> i p", i=8))
                    ps = tps.tile([128, 8], FP, tag="tp")
                    nc.tensor.transpose(ps, w8, ident_f[0:8, 0:8])
                    t = rows.tile([128, 8], FP, name=f"ln1scat{i}")
                    nc.vector.tensor_copy(out=t, in_=ps)
                    ln1scat.append(t)
                gam1_s, sh1_s = ln1scat
                sh1_sb = rows.tile([128, 8], BF)
                nc.vector.tensor_copy(out=sh1_sb, in_=sh1_s)

            gam2_s = rows.tile([128, 8], FP)
            nc.vector.tensor_scalar_add(gam2_s, scat_scmlp, 1.0)
            nc.vector.tensor_mul(gam2_s, gam2_s, n2w_sb)

            # ---------------- xslice^T (residual, bf16, via PE) --------
            xsT = [x2p.tile([128, TOK], BF, name=f"xsT{j}") for j in range(8)]
            with tc.tile_pool(name="xsp", bufs=2) as xsp:
                for ti in range(2):
                    xsb = xsp.tile([128, D], BF, tag="xsb")
                    nc.sync.dma_start(out=xsb,
                                      in_=xs_d[128 * ti:128 * ti + 128])
                    for fj in range(8):
                        ps = tps.tile([128, 128], BF, tag="tp")
                        nc.tensor.transpose(
                            ps, xsb[:, 128 * fj:128 * fj + 128], ident_b)
                        nc.scalar.copy(
                            out=xsT[fj][:, 128 * ti:128 * ti + 128], in_=ps)

            # ---------------- phase 1+2: LN1 -> h^T -> QKV -> RoPE -----
            qT = qkvr.tile([128, S], BF)
            kT = qkvr.tile([128, S], BF)
            vT = qkvr.tile([128, S], BF)
            qkv_dst = [qT, kT, vT]
            h_dr = [dram.tile([N, D], BF, name=f"hdr{hh}") for hh in range(2)]

            with tc.tile_pool(name="hTp", bufs=1) as hTp, \
                 tc.tile_pool(name="xp", bufs=2) as xp, \
                 tc.tile_pool(name="statp", bufs=4) as statp, \
                 tc.tile_pool(name="wqkvp", bufs=1) as wqkvp, \
                 tc.tile_pool(name="ropep", bufs=3) as ropep, \
                 tc.tile_pool(name="qkvps", bufs=2, space="PSUM") as qkvps:

                x_r = x_d.rearrange("(t p) d -> p t d", p=128)  # [128,16,D]
                for g in range(4):
                    xg = xp.tile([128, 4, D], BF, tag="x")
                    nc.sync.dma_start(out=xg, in_=x_r[:, 4 * g:4 * g + 4, :])
                    hg = xp.tile([128, 4, D], BF, tag="h")
                    for sub in range(4):
                        ti = 4 * g + sub
                        x_sb = xg[:, sub, :]
                        st = statp.tile([128, 2, 6], FP, tag="bst")
                        for sg in range(2):
                            nc.vector.bn_stats(
                                out=st[:, sg, :],
                                in_=x_sb[:, 512 * sg:512 * sg + 512])
                        mv = statp.tile([128, 2], FP, tag="mv")
                        nc.vector.bn_aggr(out=mv, in_=st)
                        sd = statp.tile([128, 1], FP, tag="sd")
                        nc.scalar.activation(out=sd, in_=mv[:, 1:2],
                                             func=AF.Sqrt, bias=eps128,
                                             scale=1.0)
                        rstd = statp.tile([128, 1], FP, tag="rstd")
                        nc.vector.reciprocal(out=rstd, in_=sd)
                        nmr = statp.tile([128, 1], FP, tag="nmr")
                        # -mu * rstd (ACT bias); hhat = rstd*x - mu*rstd
                        nc.vector.tensor_mul(nmr, mv[:, 0:1], rstd)
                        nc.vector.tensor_scalar_mul(nmr, nmr, -1.0)
                        h_bf = hg[:, sub, :]
                        nc.scalar.activation(out=h_bf, in_=x_sb,
                                             func=AF.Identity,
                                             bias=nmr, scale=rstd)
                    half = g // 2
                    rows0 = 512 * (g % 2)
                    nc.sync.dma_start(
                        out=h_dr[half].rearrange(
                            "(t p) d -> p t d", p=128)[:, 4 * (g % 2):4 * (g % 2) + 4, :],
                        in_=hg)

                # h^T: per (feature-tile, half) DMA transpose [1024,128]->[128,1024]
                hT = [[hTp.tile([128, N], BF, name=f"hT{k}_{hh}")
                       for hh in range(2)] for k in range(8)]
                for fj in range(8):
                    for hh in range(2):
                        nc.sync.dma_start_transpose(
                            out=hT[fj][hh],
                            in_=h_dr[hh][:, 128 * fj:128 * fj + 128])

                wq_sb = [wqkvp.tile([128, 8 * 128], BF, name=f"wq{m}")
                         for m in range(3)]
                for m in range(3):
                    nc.scalar.dma_start(
                        out=wq_sb[m].rearrange("p (k c) -> p k c", c=128),
                        in_=wqkv_d[m])
                # beta @ W^T (uses unscaled W), then scale W by gamma in place
                bw_dr = dram.tile([3, 128], FP)
                with tc.tile_pool(name="bwp", bufs=1) as bwp, \
                     tc.tile_pool(name="bwps", bufs=2, space="PSUM") as bwps:
                    bw_ps = bwps.tile([1, 384], FP)
                    for m in range(3):
                        for k in range(8):
                            nc.tensor.matmul(
                                bw_ps[:, 128 * m:128 * m + 128],
                                sh1_sb[:, k:k + 1],
                                wq_sb[m][:, 128 * k:128 * k + 128],
                                start=(k == 0), stop=(k == 7),
                                skip_group_check=True)
                    bw_row = bwp.tile([1, 384], FP)
                    nc.vector.tensor_copy(out=bw_row, in_=bw_ps)
                    nc.sync.dma_start(
                        out=bw_dr.rearrange("a b -> (a b)")[None, :],
                        in_=bw_row)
                    for m in range(3):
                        for k in range(8):
                            nc.vector.tensor_scalar_mul(
                                wq_sb[m][:, 128 * k:128 * k + 128],
                                wq_sb[m][:, 128 * k:128 * k + 128],
                                gam1_s[:, k:k + 1])
                    bw8 = bwp.tile([3, 128], FP)
                    nc.sync.dma_start(out=bw8, in_=bw_dr)
                    ps = tps.tile([128, 3], FP, tag="tp")
                    nc.tensor.transpose(ps, bw8, ident_f[0:3, 0:3])
                    bw_scat = rows.tile([128, 3], FP)
                    nc.vector.tensor_copy(out=bw_scat, in_=ps)

                for n in range(4):
                    hh = n // 2
                    nsl = slice(512 * (n % 2), 512 * (n % 2) + 512)
                    pos = slice(512 * (n % 2), 512 * (n % 2) + 512)
                    for m in range(3):
                        ps = qkvps.tile([128, 512], FP, tag="qkvps")
                        for k in range(8):
                            nc.tensor.matmul(
                                ps, wq_sb[m][:, 128 * k:128 * k + 128],
                                hT[k][hh][:, nsl],
                                start=(k == 0), stop=(k == 7))
                        # rope on bf16; ACT evac adds the beta@W^T bias
                        pb = ropep.tile([128, 512], BF, tag="pb")
                        nc.scalar.activation(out=pb, in_=ps, func=AF.Identity,
                                             bias=bw_scat[:, m:m + 1],
                                             scale=1.0)
                        t1 = ropep.tile([128, 512], BF, tag="t1")
                        nc.vector.tensor_mul(t1, pb, cos_sb[:, pos])
                        # sin table is stored pre-swapped so in0/in1 share a
                        # base partition (walrus: both-SB inputs must align);
                        # only the OUTPUT lands on the swapped half.
                        t2 = ropep.tile([128, 512], BF, tag="t2")
                        for h in range(2):
                            r = 64 * h
                            nc.vector.tensor_mul(
                                t2[r:r + 32, :], pb[r + 32:r + 64, :],
                                sin_sb[r + 32:r + 64, pos])
                            nc.vector.tensor_mul(
                                t2[r + 32:r + 64, :], pb[r:r + 32, :],
                                sin_sb[r:r + 32, pos])
                        nc.vector.tensor_add(
                            qkv_dst[m][:, 512 * n:512 * n + 512], t1, t2)

            # ---------------- phase 3: V token-major (+ones col) -------
            vaug = [vaugp.tile([128, 130], BF, name=f"vaug{kt}")
                    for kt in range(16)]
            for kt in range(16):
                ps = tps.tile([128, 128], BF, tag="tp")
                nc.tensor.transpose(
                    ps, vT[:, 128 * kt:128 * kt + 128], ident_b)
                va = vaug[kt]
                nc.vector.memset(va[:, 64:65], 1.0)
                nc.vector.memset(va[:, 129:130], 1.0)
                nc.scalar.copy(
                    out=va[:, 0:130].rearrange(
                        "p (h y) -> p h y", y=65)[:, :, 0:64],
                    in_=ps.rearrange("p (h d) -> p h d", d=64))

            # ---------------- phase 4: sparse attention ----------------
            # onorm split by q-half (precise deps); ONE AllToAll ships both
            # halves (two collectives in one NEFF crash NRT). Core j owns the
            # contiguous token block [256j, 256j+256).
            onorm = [qkvr.tile([128, N], BF, name=f"onorm{hh}")
                     for hh in range(2)]
            obounce = dram.tile([NCORES, 128, TOK], BF)
            orecvb = dram.tile([NCORES, 128, TOK], BF)
            orecv = x2p.tile([128, 8 * TOK], BF)
            with tc.tile_pool(name="sps", bufs=3, space="PSUM") as sps, \
                 tc.tile_pool(name="ops", bufs=2, space="PSUM") as ops, \
                 tc.tile_pool(name="ptp", bufs=4) as ptp, \
                 tc.tile_pool(name="nrm", bufs=2) as nrm:
                for c in range(4):
                    items = sched[c]
                    q0 = 512 * c
                    for h in range(2):
                        o_ps = ops.tile([65, 512], FP, tag="ops")
                        for idx, (kt, c0, c1, mk) in enumerate(items):
                            w = c1 - c0
                            s_ps = sps.tile([128, w], FP, tag="sps")
                            nc.tensor.matmul(
                                s_ps,
                                kT[64 * h:64 * h + 64,
                                   128 * kt:128 * kt + 128],
                                qT[64 * h:64 * h + 64, q0 + c0:q0 + c1],
                                start=True, stop=True)
                            p_sb = ptp.tile([128, w], BF, tag="pt")
                            nc.scalar.activation(out=p_sb, in_=s_ps,
                                                 func=AF.Exp, scale=0.125)
                            if mk is not None:
                                mo = MASK_OFF[mk]
                                nc.gpsimd.tensor_mul(
                                    p_sb[:, 0:128], p_sb[:, 0:128],
                                    mask_sb[:, mo:mo + 128])
                            nc.tensor.matmul(
                                o_ps[:, c0:c1],
                                vaug[kt][:, 65 * h:65 * h + 65],
                                p_sb, start=(idx == 0),
                                stop=(idx == len(items) - 1),
                                skip_group_check=True)
                        recip = nrm.tile([1, 512], FP, tag="recip")
                        nc.vector.reciprocal(out=recip, in_=o_ps[64:65, :])
                        rbc = nrm.tile([64, 512], FP, tag="rbc")
                        nc.gpsimd.partition_broadcast(rbc, recip)
                        nc.vector.tensor_mul(
                            onorm[c // 2][64 * h:64 * h + 64,
                                          q0 % N:q0 % N + 512],
                            o_ps[0:64, :], rbc)
                    # after both q-chunks of a half are done, stage that half
                    # of the bounce buffer (chunks j<4 from half 0, j>=4
                    # from half 1):
                    if c % 2 == 1:
                        hh = c // 2
                        nc.sync.dma_start(
                            out=obounce[4 * hh:4 * hh + 4].rearrange(
                                "j p t -> p j t"),
                            in_=onorm[hh].rearrange("p (j t) -> p j t", t=TOK))

            if single:
                nc.sync.dma_start(out=orecvb[:], in_=obounce[:])
            else:
                nc.gpsimd.collective_compute(
                    "AllToAll", ALU.bypass,
                    replica_groups=[list(range(NCORES))],
                    ins=[obounce.opt()], outs=[orecvb.opt()])
            nc.sync.dma_start(
                out=orecv.rearrange("p (j t) -> p j t", t=TOK),
                in_=orecvb.rearrange("j p t -> p j t"))

            # ---------------- phase 6: attn_out + residual -------------
            x2T = [x2p.tile([128, TOK], FP, name=f"x2T{m}") for m in range(8)]
            x2b = [x2p.tile([128, TOK], BF, name=f"x2b{m}") for m in range(8)]
            sqb = [x2p.tile([128, TOK], BF, name=f"sqb{m}") for m in range(8)]
            with tc.tile_pool(name="waop", bufs=2) as waop, \
                 tc.tile_pool(name="aops", bufs=2, space="PSUM") as aops:
                for g in range(2):
                    waog = waop.tile([128, 4 * 8 * 128], BF, tag="wao")
                    nc.scalar.dma_start(
                        out=waog.rearrange("p (mi k c) -> p mi k c",
                                           k=8, c=128),
                        in_=wao_d[g])
                    for mi in range(4):
                        m = 4 * g + mi
                        ps = aops.tile([128, TOK], FP, tag="aops")
                        for k in range(8):
                            off = 1024 * mi + 128 * k
                            nc.tensor.matmul(
                                ps, waog[:, off:off + 128],
                                orecv[:, TOK * k:TOK * k + TOK],
                                start=(k == 0), stop=(k == 7))
                        nc.vector.scalar_tensor_tensor(
                            out=x2T[m], in0=ps, scalar=scat_gmsa[:, m:m + 1],
                            in1=xsT[m], op0=ALU.mult, op1=ALU.add)
                        nc.gpsimd.tensor_copy(out=x2b[m], in_=x2T[m])
                        nc.gpsimd.tensor_mul(sqb[m], x2b[m], x2b[m])

            # ---------------- phase 7: LN2 + modulate ------------------
            h2T = [x2p.tile([128, TOK], BF, name=f"h2T{k}") for k in range(8)]
            with tc.tile_pool(name="l2ps", bufs=2, space="PSUM") as l2ps, \
                 tc.tile_pool(name="l2t", bufs=1) as l2t:
                sum_ps = l2ps.tile([1, TOK], FP, tag="l2sum")
                for k in range(8):
                    nc.tensor.matmul(sum_ps, ones_sb, x2b[k],
                                     start=(k == 0), stop=(k == 7))
                ssq_ps = l2ps.tile([1, TOK], FP, tag="l2ssq")
                for k in range(8):
                    nc.tensor.matmul(ssq_ps, ones_sb, sqb[k],
                                     start=(k == 0), stop=(k == 7),
                                     skip_group_check=True)
                mu2 = l2t.tile([1, TOK], FP)
                nc.vector.tensor_scalar_mul(mu2, sum_ps, 1.0 / D)
                var2 = l2t.tile([1, TOK], FP)
                musq = l2t.tile([1, TOK], FP)
                nc.vector.tensor_mul(musq, mu2, mu2)
                nc.vector.tensor_scalar_mul(var2, ssq_ps, 1.0 / D)
                nc.vector.tensor_sub(var2, var2, musq)
                sd2 = l2t.tile([1, TOK], FP)
                nc.scalar.activation(out=sd2, in_=var2, func=AF.Sqrt,
                                     bias=eps1, scale=1.0)
                rstd2 = l2t.tile([1, TOK], FP)
                nc.vector.reciprocal(out=rstd2, in_=sd2)
                mu2bc = l2t.tile([128, TOK], FP)
                nc.gpsimd.partition_broadcast(mu2bc, mu2)
                rstd2bc = l2t.tile([128, TOK], FP)
                nc.gpsimd.partition_broadcast(rstd2bc, rstd2)
                for k in range(8):
                    u = l2t.tile([128, TOK], FP, tag="u", bufs=2)
                    nc.vector.tensor_sub(u, x2T[k], mu2bc)
                    nc.vector.scalar_tensor_tensor(
                        out=h2T[k], in0=u, scalar=gam2_s[:, k:k + 1],
                        in1=rstd2bc, op0=ALU.mult, op1=ALU.mult)
                    nc.vector.tensor_scalar_add(
                        h2T[k], h2T[k], scat_shmlp[:, k:k + 1])

            # ---------------- phase 8: MLP -----------------------------
            g_sb = [gp.tile([128, TOK], BF, name=f"g{m}") for m in range(32)]
            with tc.tile_pool(name="w1p", bufs=3) as w1p, \
                 tc.tile_pool(name="m1ps", bufs=3, space="PSUM") as m1ps:
                for g in range(8):
                    w1g = w1p.tile([128, 4 * 8 * 128], BF, tag="w1")
                    nc.scalar.dma_start(
                        out=w1g.rearrange("p (mi k c) -> p mi k c",
                                          k=8, c=128),
                        in_=w1_d[g])
                    for mi in range(4):
                        m = 4 * g + mi
                        ps = m1ps.tile([128, TOK], FP, tag="m1")
                        for k in range(8):
                            off = 1024 * mi + 128 * k
                            nc.tensor.matmul(ps, w1g[:, off:off + 128],
                                             h2T[k],
                                             start=(k == 0), stop=(k == 7))
                        gfunc = (AF.Identity if os.environ.get("DBG_NO_GELU")
                                 else AF.Gelu_apprx_tanh)
                        nc.scalar.activation(out=g_sb[m], in_=ps,
                                             func=gfunc,
                                             bias=b1_sb[:, m:m + 1],
                                             scale=1.0)

            outT = [x2p.tile([128, TOK], FP, name=f"outT{m}")
                    for m in range(8)]
            with tc.tile_pool(name="w2p", bufs=2) as w2p, \
                 tc.tile_pool(name="m2ps", bufs=2, space="PSUM") as m2ps:
                for m in range(8):
                    w2m = w2p.tile([128, 32 * 128], BF, tag="w2")
                    nc.scalar.dma_start(
                        out=w2m.rearrange("p (k c) -> p k c", c=128),
                        in_=w2_d[m])
                    ps = m2ps.tile([128, TOK], FP, tag="m2")
                    for k in range(32):
                        nc.tensor.matmul(ps, w2m[:, 128 * k:128 * k + 128],
                                         g_sb[k],
                                         start=(k == 0), stop=(k == 31))
                    nc.vector.tensor_scalar(
                        out=outT[m], in0=ps, scalar1=b2_sb[:, m:m + 1],
                        scalar2=scat_gmlp[:, m:m + 1],
                        op0=ALU.add, op1=ALU.mult)
                    nc.vector.tensor_add(outT[m], outT[m], x2T[m])

            # ---------------- phase 9: transpose to token-major --------
            with tc.tile_pool(name="otm", bufs=2) as otm:
                for ti in range(2):
                    ot = otm.tile([128, D], FP, tag="otm")
                    for fj in range(8):
                        ps = tps.tile([128, 128], FP, tag="tp")
                        nc.tensor.transpose(
                            ps, outT[fj][:, 128 * ti:128 * ti + 128],
                            ident_f)
                        nc.scalar.copy(out=ot[:, 128 * fj:128 * fj + 128],
                                       in_=ps)
                    nc.sync.dma_start(out=out_d[128 * ti:128 * ti + 128],
                                      in_=ot)

    nc.compile()
    return nc


# ---------------------------------------------------------------------------
# host side
# ---------------------------------------------------------------------------

_NC = None


def _get_nc():
    global _NC
    if _NC is None:
        _NC = build_program()
    return _NC


def _mask01_tiles():
    """[128,128] multiplicative 0/1 masks in S^T orientation (rows=k,
    cols=q), concatenated [diag | strict | incl]."""
    a = np.arange(128) // BS
    diag = (a[:, None] == a[None, :])
    strict = (a[None, :] > a[:, None])
    incl = (a[None, :] >= a[:, None])
    m = np.concatenate([diag, strict, incl], axis=1).astype(np.float32)
    return np.ascontiguousarray(m.astype(bf16))


def _tile4(wT, km, mm):
    """[K, M] -> (m, p, k, c) with arr[m, p, k, c] = wT[128k+p, 128m+c]."""
    return wT.reshape(km, 128, mm, 128).transpose(2, 1, 0, 3)


def _group(w4, gs):
    """(m, p, k, c) -> (g, p, m_in_g, k, c) groups of gs m-tiles."""
    mm, p, km, c = w4.shape
    return np.ascontiguousarray(
        w4.reshape(mm // gs, gs, p, km, c).transpose(0, 2, 1, 3, 4)
        .astype(bf16))


def _prep_inputs(x, c, cos, sin, norm1_w, qkv_w, attn_out_w, norm2_w,
                 mlp_w1, mlp_b1, mlp_w2, mlp_b2, adaLN_w, adaLN_b):
    f32 = np.float32
    x = np.ascontiguousarray(
        np.asarray(x, f32).reshape(S, D).astype(bf16))
    c = np.asarray(c, f32).reshape(COND)
    cos = np.asarray(cos, f32)
    sin = np.asarray(sin, f32)

    cs = np.concatenate([cos, cos], axis=-1).T  # [64, N]
    # swapped-sign layout: rows 0:32 hold +sin (read for the d>=32 output
    # half), rows 32:64 hold -sin (read for the d<32 output half)
    ss = np.concatenate([sin.T, -sin.T], axis=0)  # [64, N]
    trig = np.ascontiguousarray(np.hstack([
        np.vstack([cs, cs]), np.vstack([ss, ss])]).astype(bf16))  # [128, 2N]

    waoT = _group(_tile4(np.asarray(attn_out_w, f32).T, 8, 8), 4)
    w1T = _group(_tile4(np.asarray(mlp_w1, f32).T, 8, 32), 4)
    w2T = np.ascontiguousarray(
        _tile4(np.asarray(mlp_w2, f32).T, 32, 8).astype(bf16))
    adawT = np.ascontiguousarray(np.asarray(adaLN_w, f32).T.astype(bf16))
    adab = np.asarray(adaLN_b, f32).reshape(6 * D)
    adab_sh = np.ascontiguousarray(adab[0:D].reshape(1, D))
    adab_sc = np.ascontiguousarray(adab[D:2 * D].reshape(1, D))
    adab_scat = np.hstack(
        [adab[(2 + i) * D:(3 + i) * D].reshape(8, 128).T for i in range(4)])
    n1w = np.asarray(norm1_w, f32).reshape(1, D)
    smallc = np.ascontiguousarray(np.hstack([
        np.asarray(norm2_w, f32).reshape(8, 128).T,
        np.asarray(mlp_b1, f32).reshape(32, 128).T,
        np.asarray(mlp_b2, f32).reshape(8, 128).T,
        adab_scat]).astype(f32))  # [128, 80]

    qkv_w = np.asarray(qkv_w, f32)
    common = {
        "x": x, "cvec": np.ascontiguousarray(c[:, None]),
        "waoT": waoT, "w1T": w1T, "w2T": w2T,
        "adawT": adawT, "adab_sh": adab_sh, "adab_sc": adab_sc,
        "n1w": n1w, "smallc": smallc, "trig": trig,
        "mask01": _mask01_tiles(),
    }
    in_maps = []
    for j in range(NCORES):
        wq = np.stack([
            np.ascontiguousarray(
                qkv_w[s * D + 128 * j: s * D + 128 * j + 128].T
                .reshape(8, 128, 128))
            for s in range(3)])  # [3, k, p, c]
        wq = np.ascontiguousarray(wq.transpose(0, 2, 1, 3).astype(bf16))
        m = dict(common)
        m["wqkvT"] = wq  # [3, 128, 8, 128] = (s, p, k, c)
        m["xslice"] = np.ascontiguousarray(x[TOK * j:TOK * j + TOK])
        in_maps.append(m)
    return in_maps


def kernel(**inputs):
    nc = _get_nc()
    in_maps = _prep_inputs(**inputs)
    res = run_bass_kernel_spmd(nc, in_maps, core_ids=list(range(NCORES)))
    out = np.concatenate([res.results[j]["out"] for j in range(NCORES)],
                         axis=0)
    return out.reshape(1, S, D).astype(np.float32)

